# revision 7
# baseline (speedup 1.0000x reference)
"""Bass/Tile kernel for nn_MultiHeadAttention (B=2, S=2048, D=1024, H=16) on 8 trn2 cores.

Sharding: core c -> (b = c//4, head-group hg = c%4). Each core computes 4 heads'
q/k/v projections, relu-attention, and a partial FC (256 of 1024 contraction rows).
Host pre-casts to bf16, pre-transposes x / weight slices, and sums the 4
partials per batch + bias.

v5 design (statically woven schedule, PE-order enforced):
  - scores: 2x row-tiled concurrent K=64 pairs; attn@v: 2x col-tiled M=64 pairs
  - PSUM evacuation (relu fp32 PSUM read, 1 elem/cy/partition/engine) is the
    co-bottleneck: each (qb,hp) window weaves score pairs with filler PE work
    (k/v/q/fc groups + lagged av pairs) so the PE stays busy at the pace the
    DVE+ACT drain allows, while mode switches stay batched (~10/window)
  - every tensor-engine matmul is chained with add_dep_helper so the Tile
    scheduler cannot reorder the stream (v4 showed it reintroduces row/col
    mode thrash, ~110ns per switch)
  - evac alternates ACT (172+FD cyc @1.2GHz) / DVE (120+FD @0.96GHz), one
    [P,2,512] paired relu per scores pair
  - y output in bf16, host sums partials
"""
import numpy as np
import ml_dtypes

import concourse.bass as bass
import concourse.mybir as mybir
import concourse.tile as tile
from concourse.tile import add_dep_helper

F32 = mybir.dt.float32
BF16 = mybir.dt.bfloat16
ts, ds = bass.ts, bass.ds

S = 2048
D = 1024
DL = 256      # per-core q/k/v dim (4 heads x 64)
P = 128
KD = D // P   # 8 k-chunks for projections
SQ = 512      # q-block (matmul N)
NQB = S // SQ # 4
NM = S // P   # 16 kpos chunks
DLC = DL // P # 2


def split_excess_waits(nc, max_embed: int = 1):
    """walrus core_v3 codegen accepts at most one sync-wait per instruction;
    move extra waits onto standalone event-sem instructions inserted before."""
    n_split = 0
    counter = 0
    for f in nc.m.functions:
        for blk in f.blocks:
            insts = blk.instructions
            if not any(
                ins.sync_info is not None and len(ins.sync_info.on_wait) > max_embed
                for ins in insts
            ):
                continue
            newl = []
            for ins in insts:
                si = ins.sync_info
                if si is not None and len(si.on_wait) > max_embed:
                    waits = list(si.on_wait)
                    extra, keep = waits[:-max_embed], waits[-max_embed:]
                    for w in extra:
                        counter += 1
                        es = mybir.InstEventSemaphore(name=f"waitsplit_{counter}")
                        es.engine = ins.engine
                        es.sync_info = mybir.SyncInfo(on_wait=[w], on_update=[])
                        newl.append(es)
                        n_split += 1
                    si.on_wait = keep
                newl.append(ins)
            blk.instructions = newl
    return n_split


def build_nc(with_mask: bool):
    nc = bass.Bass()
    # pre-arranged on host: x[p, c, s] = x.T[128c+p, s]; w[p, c, f] = w.T[128c+p, f]
    xT = nc.dram_tensor("xT", [P, KD, S], BF16, kind="ExternalInput")
    wq = nc.dram_tensor("wq", [P, KD, DL], BF16, kind="ExternalInput")
    wk = nc.dram_tensor("wk", [P, KD, DL], BF16, kind="ExternalInput")
    wv = nc.dram_tensor("wv", [P, KD, DL], BF16, kind="ExternalInput")
    wfc = nc.dram_tensor("wfc", [P, DLC, D], BF16, kind="ExternalInput")
    maskT = nc.dram_tensor("maskT", [S, S], F32, kind="ExternalInput") if with_mask else None
    y = nc.dram_tensor("y", [S, D], BF16, kind="ExternalOutput")

    with tile.TileContext(nc) as tc:
        _Emitter(tc, xT, wq, wk, wv, wfc, maskT, y).run()
    split_excess_waits(nc)
    return nc


class _Emitter:
    def __init__(self, tc, xT, wq, wk, wv, wfc, maskT, y):
        self.tc = tc
        self.nc = tc.nc
        self.xT, self.wq, self.wk, self.wv, self.wfc = xT, wq, wk, wv, wfc
        self.maskT, self.y = maskT, y
        self.ev = 0
        self.dq = 0
        self.prev_mm = None

    # -- engine helpers -----------------------------------------------------
    def mm(self, out_ap, lhsT, rhs, **kw):
        r = self.nc.tensor.matmul(out_ap, lhsT, rhs, **kw)
        ins = getattr(r, "ins", r)
        if self.prev_mm is not None:
            add_dep_helper(ins, self.prev_mm, reason="static-pe-order")
        self.prev_mm = ins
        return r

    def dma(self, out_ap, in_ap):
        eng = (self.nc.sync, self.nc.gpsimd)[self.dq % 2]
        eng.dma_start(out_ap, in_ap)
        self.dq += 1

    def evac(self, out_ap, in_ap, relu: bool):
        """PSUM->SBUF drain, alternating ACT / DVE (ACT is faster on PSUM)."""
        use_act = self.ev % 2 == 0
        self.ev += 1
        if relu:
            if use_act:
                self.nc.scalar.activation(out_ap, in_ap, mybir.ActivationFunctionType.Relu)
            else:
                self.nc.vector.tensor_scalar_max(out_ap, in_ap, 0.0)
        else:
            if use_act:
                self.nc.scalar.copy(out_ap, in_ap)
            else:
                self.nc.vector.tensor_copy(out_ap, in_ap)

    # -- emission pieces ----------------------------------------------------
    def qp0(self):
        """q-projection block 0, both c-chunks, as a 2-bank pair (runs upfront)"""
        nc = self.nc
        pt = self.ps.tile([P, 2, SQ], F32, tag="sc", name="qp0")
        for c in range(DLC):
            for k in range(KD):
                self.mm(
                    pt[:, c, :], self.wq_sb[:, k, ts(c, P)], self.xb[:, k, ds(0, SQ)],
                    start=(k == 0), stop=(k == KD - 1),
                )
        self.evac(self.qT[:, :, ds(0, SQ)], pt[:, :, :], relu=False)

    def kq_group(self, wsb, dstT, c, nb):
        """single-bank projection group: dstT[:, c, nb-block] via 8 matmuls"""
        nc = self.nc
        pt = self.ps.tile([P, SQ], F32, tag="fl", bufs=1, name=f"pj_{dstT.name}_{c}_{nb}")
        for k in range(KD):
            self.mm(
                pt[:], wsb[:, k, ts(c, P)], self.xb[:, k, ds(nb * SQ, SQ)],
                start=(k == 0), stop=(k == KD - 1),
            )
        self.evac(dstT[:, c, ds(nb * SQ, SQ)], pt[:], relu=False)

    def v_pair(self, sp):
        """two kpos-chunks of the v projection in one bank, paired copyback"""
        nc = self.nc
        pt = self.ps.tile([P, SQ], F32, tag="fl", bufs=1, name=f"v_{sp}")
        ptv = pt[:].rearrange("p (a b) -> p a b", a=2)
        for j in range(2):
            sc = 2 * sp + j
            for k in range(KD):
                self.mm(
                    ptv[:, j, :], self.xb[:, k, ts(sc, P)], self.wv_sb[:, k, :],
                    start=(k == 0), stop=(k == KD - 1),
                )
        self.evac(self.vN[:, ds(2 * sp, 2), :], ptv[:, :, :], relu=False)

    def fc_group(self, sc, eb):
        """single-bank fc group: y rows [sc] cols [eb-half]"""
        nc = self.nc
        pt = self.ps.tile([P, SQ], F32, tag="fl", bufs=1, name=f"fc_{sc}_{eb}")
        for c in range(DLC):
            self.mm(
                pt[:], self.outT[:, c, ts(sc, P)], self.wfc_sb[:, c, ds(eb * SQ, SQ)],
                start=(c == 0), stop=(c == DLC - 1),
            )
        if eb == 0:
            self.cur_yt = self.ystage.tile([P, D], BF16, tag="yt", name=f"yt_{sc}")
        self.evac(self.cur_yt[:, ds(eb * SQ, SQ)], pt[:], relu=False)
        if eb == 1:
            self.nc.sync.dma_start(self.y[ts(sc, P), :], self.cur_yt[:])

    def scores_pair(self, qb, hp, m):
        """2x row-tiled concurrent K=64 matmuls + one paired relu"""
        nc = self.nc
        pt = self.ps.tile([P, 2, SQ], F32, tag="sc", name=f"sc_{qb}_{hp}_{m}")
        for h in range(2):
            self.mm(
                pt[:, h, :],
                self.kT[ds(64 * h, 64), hp, ts(m, P)],
                self.qT[ds(64 * h, 64), hp, ds(qb * SQ, SQ)],
                start=True, stop=True,
                tile_position=(64 * h, 0),
            )
        if self.cur_mask is not None:
            for h in range(2):
                nc.vector.tensor_tensor(
                    pt[:, h, :], pt[:, h, :], self.cur_mask[:, m, :], mybir.AluOpType.add
                )
        self.evac(self.cur_at[:, m, :, :], pt[:, :, :], relu=True)

    def av_pair(self, qb, hp, m):
        """2x col-tiled concurrent M=64 matmuls, chained over m"""
        for h in range(2):
            self.mm(
                self.cur_po[ds(64 * h, 64), :],
                self.vN[:, m, ds(128 * hp + 64 * h, 64)],
                self.cur_at[:, m, h, :],
                start=(m == 0), stop=(m == NM - 1),
                tile_position=(0, 64 * h),
            )

    def load_mask(self, qb):
        if self.maskT is None:
            return None
        nc = self.nc
        mtile = self.mstg.tile([P, NM, SQ], F32, tag="mask", name=f"mask_{qb}")
        for m in range(NM):
            nc.gpsimd.dma_start(
                mtile[:, m, :],
                self.maskT[:, :].rearrange("(m p) q -> p m q", p=P)[:, m, ds(qb * SQ, SQ)],
            )
        return mtile

    # -- window drivers -----------------------------------------------------
    def open_window(self, qb, hp):
        self.cur_at = self.attn_pool.tile(
            [P, NM, 2, SQ], BF16, tag="attn", name=f"attn_{qb}_{hp}"
        )
        self.cur_po = self.ps.tile([P, SQ], F32, tag="av", bufs=1, name=f"av_{qb}_{hp}")

    def close_window(self, qb, hp):
        self.evac(self.outT[:, hp, ds(qb * SQ, SQ)], self.cur_po[:], relu=False)

    def window(self, qb, hp, fillers):
        """one (qb,hp) attention window: 16 score pairs woven with fillers and
        lagged av pairs. fillers: list of callables, consumed at weave points."""
        fill = list(fillers)
        self.open_window(qb, hp)
        # weave: S0-3 F S4-7 A0-3 F S8-11 A4-7 F S12-15 A8-11 F A12-15
        for m in range(4):
            self.scores_pair(qb, hp, m)
        if fill:
            fill.pop(0)()
        for m in range(4, 8):
            self.scores_pair(qb, hp, m)
        for m in range(0, 4):
            self.av_pair(qb, hp, m)
        if fill:
            fill.pop(0)()
        for m in range(8, 12):
            self.scores_pair(qb, hp, m)
        for m in range(4, 8):
            self.av_pair(qb, hp, m)
        if fill:
            fill.pop(0)()
        for m in range(12, 16):
            self.scores_pair(qb, hp, m)
        for m in range(8, 12):
            self.av_pair(qb, hp, m)
        while fill:
            fill.pop(0)()
        for m in range(12, 16):
            self.av_pair(qb, hp, m)
        self.close_window(qb, hp)

    # -- main ---------------------------------------------------------------
    def run(self):
        from contextlib import ExitStack

        tc, nc = self.tc, self.nc
        stack = ExitStack()
        sb = stack.enter_context(tc.tile_pool(name="sb", bufs=1))
        # PSUM budget (8 banks): sc pairs 3x2, filler 1x1, av 1x1
        self.ps = stack.enter_context(tc.tile_pool(name="ps", bufs=3, space="PSUM"))
        self.attn_pool = stack.enter_context(tc.tile_pool(name="attn", bufs=2))
        self.mstg = stack.enter_context(tc.tile_pool(name="mstg", bufs=2))
        self.ystage = stack.enter_context(tc.tile_pool(name="ystage", bufs=2))

        self.xb = sb.tile([P, KD, S], BF16, name="xb")
        self.wq_sb = sb.tile([P, KD, DL], BF16, name="wq_sb")
        self.wk_sb = sb.tile([P, KD, DL], BF16, name="wk_sb")
        self.wv_sb = sb.tile([P, KD, DL], BF16, name="wv_sb")
        self.wfc_sb = sb.tile([P, DLC, D], BF16, name="wfc_sb")
        self.qT = sb.tile([P, DLC, S], BF16, name="qT")
        self.kT = sb.tile([P, DLC, S], BF16, name="kT")
        self.vN = sb.tile([P, NM, DL], BF16, name="vN")
        self.outT = sb.tile([P, DLC, S], BF16, name="outT")

        # loads: wq + x-block0 first (q0 projection starts earliest), then wk,
        # x blocks 1..3, wv, wfc
        nc.gpsimd.dma_start(self.wq_sb[:], self.wq[:, :, :])
        for k in range(KD):
            nc.sync.dma_start(self.xb[:, k, ds(0, SQ)], self.xT[:, k, ds(0, SQ)])
        nc.gpsimd.dma_start(self.wk_sb[:], self.wk[:, :, :])
        for k in range(KD):
            self.dma(self.xb[:, k, ds(SQ, SQ)], self.xT[:, k, ds(SQ, SQ)])
        for k in range(KD):
            self.dma(self.xb[:, k, ds(S // 2, S // 2)], self.xT[:, k, ds(S // 2, S // 2)])
        nc.gpsimd.dma_start(self.wv_sb[:], self.wv[:, :, :])
        nc.gpsimd.dma_start(self.wfc_sb[:], self.wfc[:, :, :])

        K, Q, V, FC = self.kq_group, self.kq_group, self.v_pair, self.fc_group
        wk_sb, wq_sb = self.wk_sb, self.wq_sb

        # upfront: q block 0 (pair), k(c0, nb0)
        self.cur_mask = self.load_mask(0)
        self.qp0()
        K(wk_sb, self.kT, 0, 0)

        # qb0 hp0: weave remaining k(c0) groups, all k(c1) groups, v pairs
        self.open_window(0, 0)
        K(wk_sb, self.kT, 0, 1)
        for m in range(0, 4):
            self.scores_pair(0, 0, m)
        K(wk_sb, self.kT, 0, 2)
        for m in range(4, 8):
            self.scores_pair(0, 0, m)
        K(wk_sb, self.kT, 0, 3)
        for m in range(8, 12):
            self.scores_pair(0, 0, m)
        V(0)
        for m in range(12, 16):
            self.scores_pair(0, 0, m)
        V(1)
        for m in range(0, 4):
            self.av_pair(0, 0, m)
        K(wk_sb, self.kT, 1, 0)
        V(2)
        V(3)
        for m in range(4, 8):
            self.av_pair(0, 0, m)
        K(wk_sb, self.kT, 1, 1)
        V(4)
        V(5)
        for m in range(8, 12):
            self.av_pair(0, 0, m)
        K(wk_sb, self.kT, 1, 2)
        V(6)
        V(7)
        for m in range(12, 16):
            self.av_pair(0, 0, m)
        K(wk_sb, self.kT, 1, 3)
        self.close_window(0, 0)

        # qb0 hp1: fillers = q block 1 (both c)
        self.window(0, 1, [
            lambda: Q(wq_sb, self.qT, 0, 1),
            lambda: Q(wq_sb, self.qT, 1, 1),
        ])

        for qb in range(1, NQB):
            self.cur_mask = self.load_mask(qb)
            pf = (qb - 1) * 4  # fc seq chunks for previous qb
            for hp in range(DLC):
                fills = [
                    lambda sc=pf + 2 * hp, eb=0: FC(sc, eb),
                    lambda sc=pf + 2 * hp, eb=1: FC(sc, eb),
                    lambda sc=pf + 2 * hp + 1, eb=0: FC(sc, eb),
                    lambda sc=pf + 2 * hp + 1, eb=1: FC(sc, eb),
                ]
                if qb < NQB - 1 and hp < DLC:
                    # next q block chunk as extra filler
                    fills.insert(2, lambda c=hp, nb=qb + 1: Q(wq_sb, self.qT, c, nb))
                self.window(qb, hp, fills)

        # tail: fc for the last qb
        for sc in range(12, 16):
            for eb in range(2):
                FC(sc, eb)

        stack.close()


# ---- host wrapper ---------------------------------------------------------

N_HEAD = 16
_nc_cache = {}


def get_nc(with_mask: bool):
    if with_mask not in _nc_cache:
        _nc_cache[with_mask] = build_nc(with_mask)
    return _nc_cache[with_mask]


def make_in_maps(x, mask, Wq, Wk, Wv, Wfc, with_mask):
    scale = np.float32(1.0 / np.sqrt(D // N_HEAD))
    bf = ml_dtypes.bfloat16
    in_maps = []
    for c in range(8):
        b, hg = divmod(c, 4)
        gs = slice(DL * hg, DL * hg + DL)
        def prearrange(wT, cdim):  # [cdim*128, F] -> [128, cdim, F]
            F = wT.shape[1]
            return np.ascontiguousarray(
                wT.reshape(cdim, P, F).transpose(1, 0, 2)
            ).astype(bf)

        m = {
            "xT": prearrange(x[b].T, KD),
            "wq": prearrange((Wq[gs, :] * scale).T, KD),
            "wk": prearrange(Wk[gs, :].T, KD),
            "wv": prearrange(Wv[gs, :].T, KD),
            "wfc": prearrange(Wfc[:, gs].T, DLC),
        }
        if with_mask:
            m["maskT"] = np.ascontiguousarray(
                np.broadcast_to(mask, (1, 1, S, S))[0, 0].T.astype(np.float32)
            )
        in_maps.append(m)
    return in_maps


def kernel(x, mask, Wq, Wk, Wv, Wfc, bfc):
    """Full-input entry: shards across 8 trn2 cores, returns the full output."""
    from concourse.bass_utils import run_bass_kernel_spmd

    x = np.asarray(x, dtype=np.float32)
    mask = np.asarray(mask, dtype=np.float32)
    Wq = np.asarray(Wq, dtype=np.float32)
    Wk = np.asarray(Wk, dtype=np.float32)
    Wv = np.asarray(Wv, dtype=np.float32)
    Wfc = np.asarray(Wfc, dtype=np.float32)
    bfc = np.asarray(bfc, dtype=np.float32)

    B = x.shape[0]
    with_mask = bool(np.any(mask))
    nc = get_nc(with_mask)
    in_maps = make_in_maps(x, mask, Wq, Wk, Wv, Wfc, with_mask)

    res = run_bass_kernel_spmd(nc, in_maps, core_ids=list(range(8)))
    parts = np.stack([np.asarray(r["y"]) for r in res.results])  # [8, S, D] bf16
    out = parts.astype(np.float64).reshape(B, 4, S, D).sum(axis=1)
    out += bfc.astype(np.float64)
    return out.astype(np.float32)


# revision 12
# speedup vs baseline: 1.0341x; 1.0341x over previous
"""Bass/Tile kernel for nn_MultiHeadAttention (B=2, S=2048, D=1024, H=16) on 8 trn2 cores.

Sharding: core c -> (b = c//4, head-group hg = c%4). Each core computes 4 heads'
q/k/v projections, relu-attention, and a partial FC (256 of 1024 contraction rows).
Host pre-casts to bf16, pre-transposes x / weight slices, and sums the 4
partials per batch + bias.

v5 design (statically woven schedule, PE-order enforced):
  - scores: 2x row-tiled concurrent K=64 pairs; attn@v: 2x col-tiled M=64 pairs
  - PSUM evacuation (relu fp32 PSUM read, 1 elem/cy/partition/engine) is the
    co-bottleneck: each (qb,hp) window weaves score pairs with filler PE work
    (k/v/q/fc groups + lagged av pairs) so the PE stays busy at the pace the
    DVE+ACT drain allows, while mode switches stay batched (~10/window)
  - every tensor-engine matmul is chained with add_dep_helper so the Tile
    scheduler cannot reorder the stream (v4 showed it reintroduces row/col
    mode thrash, ~110ns per switch)
  - evac alternates ACT (172+FD cyc @1.2GHz) / DVE (120+FD @0.96GHz), one
    [P,2,512] paired relu per scores pair
  - y output in bf16, host sums partials
"""
import numpy as np
import ml_dtypes

import concourse.bass as bass
import concourse.mybir as mybir
import concourse.tile as tile
from concourse.tile import add_dep_helper

F32 = mybir.dt.float32
BF16 = mybir.dt.bfloat16
ts, ds = bass.ts, bass.ds

S = 2048
D = 1024
DL = 256      # per-core q/k/v dim (4 heads x 64)
P = 128
KD = D // P   # 8 k-chunks for projections
SQ = 512      # q-block (matmul N)
NQB = S // SQ # 4
NM = S // P   # 16 kpos chunks
DLC = DL // P # 2


def split_excess_waits(nc, max_embed: int = 1):
    """walrus core_v3 codegen accepts at most one sync-wait per instruction;
    move extra waits onto standalone event-sem instructions inserted before."""
    n_split = 0
    counter = 0
    for f in nc.m.functions:
        for blk in f.blocks:
            insts = blk.instructions
            if not any(
                ins.sync_info is not None and len(ins.sync_info.on_wait) > max_embed
                for ins in insts
            ):
                continue
            newl = []
            for ins in insts:
                si = ins.sync_info
                if si is not None and len(si.on_wait) > max_embed:
                    waits = list(si.on_wait)
                    extra, keep = waits[:-max_embed], waits[-max_embed:]
                    for w in extra:
                        counter += 1
                        es = mybir.InstEventSemaphore(name=f"waitsplit_{counter}")
                        es.engine = ins.engine
                        es.sync_info = mybir.SyncInfo(on_wait=[w], on_update=[])
                        newl.append(es)
                        n_split += 1
                    si.on_wait = keep
                newl.append(ins)
            blk.instructions = newl
    return n_split


def build_nc(with_mask: bool):
    nc = bass.Bass()
    # pre-arranged on host: x[p, c, s] = x.T[128c+p, s]; w[p, c, f] = w.T[128c+p, f]
    xT = nc.dram_tensor("xT", [P, KD, S], BF16, kind="ExternalInput")
    wq = nc.dram_tensor("wq", [P, KD, DL], BF16, kind="ExternalInput")
    wk = nc.dram_tensor("wk", [P, KD, DL], BF16, kind="ExternalInput")
    wv = nc.dram_tensor("wv", [P, KD, DL], BF16, kind="ExternalInput")
    wfc = nc.dram_tensor("wfc", [P, DLC, D], BF16, kind="ExternalInput")
    maskT = nc.dram_tensor("maskT", [S, S], F32, kind="ExternalInput") if with_mask else None
    y = nc.dram_tensor("y", [S, D], BF16, kind="ExternalOutput")

    with tile.TileContext(nc) as tc:
        _Emitter(tc, xT, wq, wk, wv, wfc, maskT, y).run()
    split_excess_waits(nc)
    return nc


class _Emitter:
    def __init__(self, tc, xT, wq, wk, wv, wfc, maskT, y):
        self.tc = tc
        self.nc = tc.nc
        self.xT, self.wq, self.wk, self.wv, self.wfc = xT, wq, wk, wv, wfc
        self.maskT, self.y = maskT, y
        self.ev = 0
        self.dq = 0
        self.prev_mm = None

    # -- engine helpers -----------------------------------------------------
    def mm(self, out_ap, lhsT, rhs, **kw):
        r = self.nc.tensor.matmul(out_ap, lhsT, rhs, **kw)
        ins = getattr(r, "ins", r)
        if self.prev_mm is not None:
            add_dep_helper(ins, self.prev_mm, reason="static-pe-order")
        self.prev_mm = ins
        return r

    def dma(self, out_ap, in_ap):
        eng = (self.nc.sync, self.nc.gpsimd)[self.dq % 2]
        eng.dma_start(out_ap, in_ap)
        self.dq += 1

    def evac(self, out_ap, in_ap, relu: bool):
        """PSUM->SBUF drain, weighted-alternating ACT / DVE.
        ACT pair evac = (172+FD)/1.2GHz, DVE = (120+FD)/0.96GHz -> ACT share ~0.545."""
        use_act = (self.ev * 6) % 11 < 6
        self.ev += 1
        if relu:
            if use_act:
                self.nc.scalar.activation(out_ap, in_ap, mybir.ActivationFunctionType.Relu)
            else:
                self.nc.vector.tensor_scalar_max(out_ap, in_ap, 0.0)
        else:
            if use_act:
                self.nc.scalar.copy(out_ap, in_ap)
            else:
                self.nc.vector.tensor_copy(out_ap, in_ap)

    # -- emission pieces ----------------------------------------------------
    def qp0(self):
        """q-projection block 0, both c-chunks, as a 2-bank pair (runs upfront)"""
        nc = self.nc
        pt = self.ps.tile([P, 2, SQ], F32, tag="sc", name="qp0")
        for c in range(DLC):
            for k in range(KD):
                self.mm(
                    pt[:, c, :], self.wq_sb[:, k, ts(c, P)], self.xb[:, k, ds(0, SQ)],
                    start=(k == 0), stop=(k == KD - 1),
                )
        self.evac(self.qT[:, :, ds(0, SQ)], pt[:, :, :], relu=False)

    def kq_group(self, wsb, dstT, c, nb):
        """single-bank projection group: dstT[:, c, nb-block] via 8 matmuls"""
        nc = self.nc
        pt = self.ps.tile([P, SQ], F32, tag="fl", bufs=1, name=f"pj_{dstT.name}_{c}_{nb}")
        for k in range(KD):
            self.mm(
                pt[:], wsb[:, k, ts(c, P)], self.xb[:, k, ds(nb * SQ, SQ)],
                start=(k == 0), stop=(k == KD - 1),
            )
        self.evac(dstT[:, c, ds(nb * SQ, SQ)], pt[:], relu=False)

    def v_group(self, m):
        """one kpos-chunk of the v projection (single bank)"""
        nc = self.nc
        pt = self.ps.tile([P, SQ], F32, tag="fl", bufs=1, name=f"v_{m}")
        for k in range(KD):
            self.mm(
                pt[:, ds(0, DL)], self.xb[:, k, ts(m, P)], self.wv_sb[:, k, :],
                start=(k == 0), stop=(k == KD - 1),
            )
        self.evac(self.vN[:, m, :], pt[:, ds(0, DL)], relu=False)

    def fc_group(self, sc, eb):
        """single-bank fc group: y rows [sc] cols [eb-half]"""
        nc = self.nc
        pt = self.ps.tile([P, SQ], F32, tag="fl", bufs=1, name=f"fc_{sc}_{eb}")
        for c in range(DLC):
            self.mm(
                pt[:], self.outT[:, c, ts(sc, P)], self.wfc_sb[:, c, ds(eb * SQ, SQ)],
                start=(c == 0), stop=(c == DLC - 1),
            )
        if eb == 0:
            self.cur_yt = self.ystage.tile([P, D], BF16, tag="yt", name=f"yt_{sc}")
        self.evac(self.cur_yt[:, ds(eb * SQ, SQ)], pt[:], relu=False)
        if eb == 1:
            self.nc.sync.dma_start(self.y[ts(sc, P), :], self.cur_yt[:])

    def scores_pair(self, qb, hp, m):
        """2x row-tiled concurrent K=64 matmuls + one paired relu"""
        nc = self.nc
        pt = self.ps.tile([P, 2, SQ], F32, tag="sc", name=f"sc_{qb}_{hp}_{m}")
        for h in range(2):
            self.mm(
                pt[:, h, :],
                self.kT[ds(64 * h, 64), hp, ts(m, P)],
                self.qT[ds(64 * h, 64), hp, ds(qb * SQ, SQ)],
                start=True, stop=True,
                tile_position=(64 * h, 0),
            )
        if self.cur_mask is not None:
            for h in range(2):
                nc.vector.tensor_tensor(
                    pt[:, h, :], pt[:, h, :], self.cur_mask[:, m, :], mybir.AluOpType.add
                )
        self.evac(self.cur_at[:, m, :, :], pt[:, :, :], relu=True)

    def av_pair(self, qb, hp, m):
        """2x col-tiled concurrent M=64 matmuls, chained over m"""
        for h in range(2):
            self.mm(
                self.cur_po[ds(64 * h, 64), :],
                self.vN[:, m, ds(128 * hp + 64 * h, 64)],
                self.cur_at[:, m, h, :],
                start=(m == 0), stop=(m == NM - 1),
                tile_position=(0, 64 * h),
            )

    def load_mask(self, qb):
        if self.maskT is None:
            return None
        nc = self.nc
        mtile = self.mstg.tile([P, NM, SQ], F32, tag="mask", name=f"mask_{qb}")
        for m in range(NM):
            nc.gpsimd.dma_start(
                mtile[:, m, :],
                self.maskT[:, :].rearrange("(m p) q -> p m q", p=P)[:, m, ds(qb * SQ, SQ)],
            )
        return mtile

    # -- window drivers -----------------------------------------------------
    def open_window(self, qb, hp):
        self.cur_at = self.attn_pool.tile(
            [P, NM, 2, SQ], BF16, tag="attn", name=f"attn_{qb}_{hp}"
        )
        self.cur_po = self.ps.tile([P, SQ], F32, tag="av", bufs=1, name=f"av_{qb}_{hp}")

    def close_window(self, qb, hp):
        self.evac(self.outT[:, hp, ds(qb * SQ, SQ)], self.cur_po[:], relu=False)

    def window(self, qb, hp, cycles):
        """one (qb,hp) attention window as 5 cycles; each cycle is
        (s_quad_idx|None, [filler callables], a_quad_idx|None). Scores bursts
        of 4 pairs, fillers (full-mode, single-bank) pace the relu drain, av
        quads run one cycle behind their scores."""
        self.open_window(qb, hp)
        for s_q, fills, a_q in cycles:
            # 3 scores pairs (matches sc bufs=3), fillers, 4th pair, av quad:
            # the filler gap lets relu of pair 0 finish before pair 3 reuses
            # its PSUM buffer
            if s_q is not None:
                for m in range(4 * s_q, 4 * s_q + 3):
                    self.scores_pair(qb, hp, m)
            for f in fills:
                f()
            if s_q is not None:
                self.scores_pair(qb, hp, 4 * s_q + 3)
            if a_q is not None:
                for m in range(4 * a_q, 4 * a_q + 4):
                    self.av_pair(qb, hp, m)
        self.close_window(qb, hp)

    # -- main ---------------------------------------------------------------
    def run(self):
        from contextlib import ExitStack

        tc, nc = self.tc, self.nc
        stack = ExitStack()
        sb = stack.enter_context(tc.tile_pool(name="sb", bufs=1))
        # PSUM budget (8 banks): sc pairs 3x2, filler 1x1, av 1x1
        self.ps = stack.enter_context(tc.tile_pool(name="ps", bufs=3, space="PSUM"))
        self.attn_pool = stack.enter_context(tc.tile_pool(name="attn", bufs=2))
        self.mstg = stack.enter_context(tc.tile_pool(name="mstg", bufs=2))
        self.ystage = stack.enter_context(tc.tile_pool(name="ystage", bufs=2))

        self.xb = sb.tile([P, KD, S], BF16, name="xb")
        self.wq_sb = sb.tile([P, KD, DL], BF16, name="wq_sb")
        self.wk_sb = sb.tile([P, KD, DL], BF16, name="wk_sb")
        self.wv_sb = sb.tile([P, KD, DL], BF16, name="wv_sb")
        self.wfc_sb = sb.tile([P, DLC, D], BF16, name="wfc_sb")
        self.qT = sb.tile([P, DLC, S], BF16, name="qT")
        self.kT = sb.tile([P, DLC, S], BF16, name="kT")
        self.vN = sb.tile([P, NM, DL], BF16, name="vN")
        self.outT = sb.tile([P, DLC, S], BF16, name="outT")

        # loads: wq + x-block0 first (q0 projection starts earliest), then wk,
        # x blocks 1..3, wv, wfc
        nc.gpsimd.dma_start(self.wq_sb[:], self.wq[:, :, :])
        for k in range(KD):
            nc.sync.dma_start(self.xb[:, k, ds(0, SQ)], self.xT[:, k, ds(0, SQ)])
        nc.gpsimd.dma_start(self.wk_sb[:], self.wk[:, :, :])
        for k in range(KD):
            self.dma(self.xb[:, k, ds(SQ, SQ)], self.xT[:, k, ds(SQ, SQ)])
        for k in range(KD):
            self.dma(self.xb[:, k, ds(S // 2, S // 2)], self.xT[:, k, ds(S // 2, S // 2)])
        nc.gpsimd.dma_start(self.wv_sb[:], self.wv[:, :, :])
        nc.gpsimd.dma_start(self.wfc_sb[:], self.wfc[:, :, :])

        wk_sb, wq_sb = self.wk_sb, self.wq_sb
        K = lambda c, nb: self.kq_group(wk_sb, self.kT, c, nb)
        Q = lambda c, nb: self.kq_group(wq_sb, self.qT, c, nb)
        V = self.v_group
        FC = self.fc_group

        def fK(c, nb):
            return lambda: K(c, nb)
        def fQ(c, nb):
            return lambda: Q(c, nb)
        def fV(m):
            return lambda: V(m)
        def fFC(sc, eb):
            return lambda: FC(sc, eb)

        # upfront: q block 0 (pair), k(c0, nb0)
        self.cur_mask = self.load_mask(0)
        self.qp0()
        K(0, 0)

        # qb0 hp0: weave remaining k(c0) groups (gate scores quads), all v
        # groups (gate av quads), and the first k(c1) groups for hp1
        self.window(0, 0, [
            (0, [fK(0, 1)], None),
            (1, [fK(0, 2), fV(0)], None),
            (2, [fK(0, 3), fV(1), fV(2), fV(3)], 0),
            (3, [fV(4), fV(5), fV(6), fV(7)], 1),
            (None, [fK(1, 0), fV(8), fV(9), fV(10), fV(11)], 2),
            (None, [fK(1, 1), fV(12), fV(13), fV(14), fV(15)], 3),
        ])

        # qb0 hp1: fillers = remaining k(c1) groups + q block 1
        self.window(0, 1, [
            (0, [fQ(0, 1)], None),
            (1, [fK(1, 2)], None),
            (2, [fK(1, 3)], 0),
            (3, [fQ(1, 1)], 1),
            (None, [], 2),
            (None, [], 3),
        ])

        for qb in range(1, NQB):
            self.cur_mask = self.load_mask(qb)
            for hp in range(DLC):
                a = (qb - 1) * 4 + 2 * hp  # fc seq chunks for previous qb
                extra = [fQ(hp, qb + 1)] if qb < NQB - 1 else []
                self.window(qb, hp, [
                    (0, [fFC(a, 0)], None),
                    (1, [fFC(a, 1)], 0),
                    (2, [fFC(a + 1, 0)], 1),
                    (3, [fFC(a + 1, 1)] + extra, 2),
                    (None, [], 3),
                ])

        # tail: fc for the last qb
        for sc in range(12, 16):
            for eb in range(2):
                FC(sc, eb)

        stack.close()


# ---- host wrapper ---------------------------------------------------------

N_HEAD = 16
_nc_cache = {}


def get_nc(with_mask: bool):
    if with_mask not in _nc_cache:
        _nc_cache[with_mask] = build_nc(with_mask)
    return _nc_cache[with_mask]


def make_in_maps(x, mask, Wq, Wk, Wv, Wfc, with_mask):
    scale = np.float32(1.0 / np.sqrt(D // N_HEAD))
    bf = ml_dtypes.bfloat16
    in_maps = []
    for c in range(8):
        b, hg = divmod(c, 4)
        gs = slice(DL * hg, DL * hg + DL)
        def prearrange(wT, cdim):  # [cdim*128, F] -> [128, cdim, F]
            F = wT.shape[1]
            return np.ascontiguousarray(
                wT.reshape(cdim, P, F).transpose(1, 0, 2)
            ).astype(bf)

        m = {
            "xT": prearrange(x[b].T, KD),
            "wq": prearrange((Wq[gs, :] * scale).T, KD),
            "wk": prearrange(Wk[gs, :].T, KD),
            "wv": prearrange(Wv[gs, :].T, KD),
            "wfc": prearrange(Wfc[:, gs].T, DLC),
        }
        if with_mask:
            m["maskT"] = np.ascontiguousarray(
                np.broadcast_to(mask, (1, 1, S, S))[0, 0].T.astype(np.float32)
            )
        in_maps.append(m)
    return in_maps


def kernel(x, mask, Wq, Wk, Wv, Wfc, bfc):
    """Full-input entry: shards across 8 trn2 cores, returns the full output."""
    from concourse.bass_utils import run_bass_kernel_spmd

    x = np.asarray(x, dtype=np.float32)
    mask = np.asarray(mask, dtype=np.float32)
    Wq = np.asarray(Wq, dtype=np.float32)
    Wk = np.asarray(Wk, dtype=np.float32)
    Wv = np.asarray(Wv, dtype=np.float32)
    Wfc = np.asarray(Wfc, dtype=np.float32)
    bfc = np.asarray(bfc, dtype=np.float32)

    B = x.shape[0]
    with_mask = bool(np.any(mask))
    nc = get_nc(with_mask)
    in_maps = make_in_maps(x, mask, Wq, Wk, Wv, Wfc, with_mask)

    res = run_bass_kernel_spmd(nc, in_maps, core_ids=list(range(8)))
    parts = np.stack([np.asarray(r["y"]) for r in res.results])  # [8, S, D] bf16
    out = parts.astype(np.float64).reshape(B, 4, S, D).sum(axis=1)
    out += bfc.astype(np.float64)
    return out.astype(np.float32)


# revision 16
# speedup vs baseline: 1.1149x; 1.0781x over previous
"""Bass/Tile kernel for nn_MultiHeadAttention (B=2, S=2048, D=1024, H=16) on 8 trn2 cores.

Sharding: core c -> (b = c//4, head-group hg = c%4). Each core computes 4 heads'
q/k/v projections, relu-attention, and a partial FC (256 of 1024 contraction rows).
Host pre-casts to bf16, pre-transposes x / weight slices, and sums the 4
partials per batch + bias.

v7 design (fine-grained woven emission, scheduler left free):
  - scores: 2x row-tiled concurrent K=64 pairs; attn@v: 2x col-tiled M=64 pairs
  - PSUM evacuation (relu fp32 PSUM read, 1 elem/cy/partition/engine) is the
    co-bottleneck: each (qb,hp) window weaves score pairs with filler PE work
    (k/v/q/fc groups + lagged av quads) at the pace the DVE+ACT drain allows
  - emission order sets scheduler priority; the weave is fine-grained enough
    that the scheduler's readiness-based gap-filling follows it (v4: coarse
    phases -> mode thrash; v6: rigid add_dep chain -> PE idled on PSUM waits)
  - evac weighted-alternates ACT (172+FD cyc @1.2GHz) / DVE (120+FD @0.96GHz),
    one [P,2,512] paired relu per scores pair
  - PSUM banks: scores pairs 2x2, fillers 3x1, av chain 1x1
  - y output in bf16 via HWDGE; weights/x on sync+scalar HWDGE queues
"""
import numpy as np
import ml_dtypes

import concourse.bass as bass
import concourse.mybir as mybir
import concourse.tile as tile
from concourse.tile import add_dep_helper

F32 = mybir.dt.float32
BF16 = mybir.dt.bfloat16
ts, ds = bass.ts, bass.ds

S = 2048
D = 1024
DL = 256      # per-core q/k/v dim (4 heads x 64)
P = 128
KD = D // P   # 8 k-chunks for projections
SQ = 512      # q-block (matmul N)
NQB = S // SQ # 4
NM = S // P   # 16 kpos chunks
DLC = DL // P # 2


def split_excess_waits(nc, max_embed: int = 1):
    """walrus core_v3 codegen accepts at most one sync-wait per instruction;
    move extra waits onto standalone event-sem instructions inserted before."""
    n_split = 0
    counter = 0
    for f in nc.m.functions:
        for blk in f.blocks:
            insts = blk.instructions
            if not any(
                ins.sync_info is not None and len(ins.sync_info.on_wait) > max_embed
                for ins in insts
            ):
                continue
            newl = []
            for ins in insts:
                si = ins.sync_info
                if si is not None and len(si.on_wait) > max_embed:
                    waits = list(si.on_wait)
                    extra, keep = waits[:-max_embed], waits[-max_embed:]
                    for w in extra:
                        counter += 1
                        es = mybir.InstEventSemaphore(name=f"waitsplit_{counter}")
                        es.engine = ins.engine
                        es.sync_info = mybir.SyncInfo(on_wait=[w], on_update=[])
                        newl.append(es)
                        n_split += 1
                    si.on_wait = keep
                newl.append(ins)
            blk.instructions = newl
    return n_split


def build_nc(with_mask: bool):
    nc = bass.Bass()
    # pre-arranged on host: x[p, c, s] = x.T[128c+p, s]; w[p, c, f] = w.T[128c+p, f]
    xT = nc.dram_tensor("xT", [P, KD, S], BF16, kind="ExternalInput")
    wq = nc.dram_tensor("wq", [P, KD, DL], BF16, kind="ExternalInput")
    wk = nc.dram_tensor("wk", [P, KD, DL], BF16, kind="ExternalInput")
    wv = nc.dram_tensor("wv", [P, KD, DL], BF16, kind="ExternalInput")
    wfc = nc.dram_tensor("wfc", [P, DLC, D], BF16, kind="ExternalInput")
    maskT = nc.dram_tensor("maskT", [S, S], F32, kind="ExternalInput") if with_mask else None
    y = nc.dram_tensor("y", [S, D], BF16, kind="ExternalOutput")

    with tile.TileContext(nc) as tc:
        _Emitter(tc, xT, wq, wk, wv, wfc, maskT, y).run()
    split_excess_waits(nc)
    return nc


class _Emitter:
    def __init__(self, tc, xT, wq, wk, wv, wfc, maskT, y):
        self.tc = tc
        self.nc = tc.nc
        self.xT, self.wq, self.wk, self.wv, self.wfc = xT, wq, wk, wv, wfc
        self.maskT, self.y = maskT, y
        self.ev = 0
        self.dq = 0
        self.prev_mm = None

    # -- engine helpers -----------------------------------------------------
    def mm(self, out_ap, lhsT, rhs, **kw):
        # emission order sets scheduler priority; with the weave fine-grained,
        # the scheduler's gap-filling follows the intended order (a rigid
        # add_dep chain measurably hurt: PE idled on PSUM waits it could fill)
        return self.nc.tensor.matmul(out_ap, lhsT, rhs, **kw)

    def dma(self, out_ap, in_ap):
        eng = (self.nc.sync, self.nc.gpsimd)[self.dq % 2]
        eng.dma_start(out_ap, in_ap)
        self.dq += 1

    def evac(self, out_ap, in_ap, relu: bool):
        """PSUM->SBUF drain, weighted-alternating ACT / DVE.
        ACT pair evac = (172+FD)/1.2GHz, DVE = (120+FD)/0.96GHz -> ACT share ~0.545."""
        use_act = (self.ev * 6) % 11 < 6
        self.ev += 1
        if relu:
            if use_act:
                self.nc.scalar.activation(out_ap, in_ap, mybir.ActivationFunctionType.Relu)
            else:
                self.nc.vector.tensor_scalar_max(out_ap, in_ap, 0.0)
        else:
            if use_act:
                self.nc.scalar.copy(out_ap, in_ap)
            else:
                self.nc.vector.tensor_copy(out_ap, in_ap)

    # -- emission pieces ----------------------------------------------------
    def qp0(self):
        """q-projection block 0, both c-chunks, as a 2-bank pair (runs upfront)"""
        nc = self.nc
        pt = self.ps.tile([P, 2, SQ], F32, tag="sc", name="qp0")
        for c in range(DLC):
            for k in range(KD):
                self.mm(
                    pt[:, c, :], self.wq_sb[:, k, ts(c, P)], self.xb[:, k, ds(0, SQ)],
                    start=(k == 0), stop=(k == KD - 1),
                )
        self.evac(self.qT[:, :, ds(0, SQ)], pt[:, :, :], relu=False)

    def kq_group(self, wsb, dstT, c, nb):
        """single-bank projection group: dstT[:, c, nb-block] via 8 matmuls"""
        nc = self.nc
        pt = self.ps.tile([P, SQ], F32, tag="fl", bufs=3, name=f"pj_{dstT.name}_{c}_{nb}")
        for k in range(KD):
            self.mm(
                pt[:], wsb[:, k, ts(c, P)], self.xb[:, k, ds(nb * SQ, SQ)],
                start=(k == 0), stop=(k == KD - 1),
            )
        self.evac(dstT[:, c, ds(nb * SQ, SQ)], pt[:], relu=False)

    def v_group(self, m):
        """one kpos-chunk of the v projection (single bank)"""
        nc = self.nc
        pt = self.ps.tile([P, SQ], F32, tag="fl", bufs=3, name=f"v_{m}")
        for k in range(KD):
            self.mm(
                pt[:, ds(0, DL)], self.xb[:, k, ts(m, P)], self.wv_sb[:, k, :],
                start=(k == 0), stop=(k == KD - 1),
            )
        self.evac(self.vN[:, m, :], pt[:, ds(0, DL)], relu=False)

    def fc_group(self, sc, eb):
        """single-bank fc group: y rows [sc] cols [eb-half]"""
        nc = self.nc
        pt = self.ps.tile([P, SQ], F32, tag="fl", bufs=3, name=f"fc_{sc}_{eb}")
        for c in range(DLC):
            self.mm(
                pt[:], self.outT[:, c, ts(sc, P)], self.wfc_sb[:, c, ds(eb * SQ, SQ)],
                start=(c == 0), stop=(c == DLC - 1),
            )
        if eb == 0:
            self.cur_yt = self.ystage.tile([P, D], BF16, tag="yt", name=f"yt_{sc}")
        self.evac(self.cur_yt[:, ds(eb * SQ, SQ)], pt[:], relu=False)
        if eb == 1:
            self.nc.sync.dma_start(self.y[ts(sc, P), :], self.cur_yt[:])

    def scores_pair(self, qb, hp, m):
        """2x row-tiled concurrent K=64 matmuls + one paired relu"""
        nc = self.nc
        pt = self.ps.tile([P, 2, SQ], F32, tag="sc", name=f"sc_{qb}_{hp}_{m}")
        for h in range(2):
            self.mm(
                pt[:, h, :],
                self.kT[ds(64 * h, 64), hp, ts(m, P)],
                self.qT[ds(64 * h, 64), hp, ds(qb * SQ, SQ)],
                start=True, stop=True,
                tile_position=(64 * h, 0),
            )
        if self.cur_mask is not None:
            for h in range(2):
                nc.vector.tensor_tensor(
                    pt[:, h, :], pt[:, h, :], self.cur_mask[:, m, :], mybir.AluOpType.add
                )
        self.evac(self.cur_at[:, m, :, :], pt[:, :, :], relu=True)

    def av_pair(self, qb, hp, m):
        """2x col-tiled concurrent M=64 matmuls, chained over m"""
        for h in range(2):
            self.mm(
                self.cur_po[ds(64 * h, 64), :],
                self.vN[:, m, ds(128 * hp + 64 * h, 64)],
                self.cur_at[:, m, h, :],
                start=(m == 0), stop=(m == NM - 1),
                tile_position=(0, 64 * h),
            )

    def load_mask(self, qb):
        if self.maskT is None:
            return None
        nc = self.nc
        mtile = self.mstg.tile([P, NM, SQ], F32, tag="mask", name=f"mask_{qb}")
        for m in range(NM):
            nc.gpsimd.dma_start(
                mtile[:, m, :],
                self.maskT[:, :].rearrange("(m p) q -> p m q", p=P)[:, m, ds(qb * SQ, SQ)],
            )
        return mtile

    # -- window drivers -----------------------------------------------------
    def open_window(self, qb, hp):
        self.cur_at = self.attn_pool.tile(
            [P, NM, 2, SQ], BF16, tag="attn", name=f"attn_{qb}_{hp}"
        )
        self.cur_po = self.ps.tile([P, SQ], F32, tag="av", bufs=1, name=f"av_{qb}_{hp}")

    def close_window(self, qb, hp):
        self.evac(self.outT[:, hp, ds(qb * SQ, SQ)], self.cur_po[:], relu=False)

    def window(self, qb, hp, cycles):
        """one (qb,hp) attention window as 5 cycles; each cycle is
        (s_quad_idx|None, [filler callables], a_quad_idx|None). Scores bursts
        of 4 pairs, fillers (full-mode, single-bank) pace the relu drain, av
        quads run one cycle behind their scores."""
        self.open_window(qb, hp)
        for s_q, fills, a_q in cycles:
            # 3 scores pairs (matches sc bufs=3), fillers, 4th pair, av quad:
            # the filler gap lets relu of pair 0 finish before pair 3 reuses
            # its PSUM buffer
            if s_q is not None:
                for m in range(4 * s_q, 4 * s_q + 3):
                    self.scores_pair(qb, hp, m)
            for f in fills:
                f()
            if s_q is not None:
                self.scores_pair(qb, hp, 4 * s_q + 3)
            if a_q is not None:
                for m in range(4 * a_q, 4 * a_q + 4):
                    self.av_pair(qb, hp, m)
        self.close_window(qb, hp)

    # -- main ---------------------------------------------------------------
    def run(self):
        from contextlib import ExitStack

        tc, nc = self.tc, self.nc
        stack = ExitStack()
        sb = stack.enter_context(tc.tile_pool(name="sb", bufs=1))
        # PSUM budget (8 banks): sc pairs 2x2, filler 3x1, av 1x1
        self.ps = stack.enter_context(tc.tile_pool(name="ps", bufs=2, space="PSUM"))
        self.attn_pool = stack.enter_context(tc.tile_pool(name="attn", bufs=2))
        self.mstg = stack.enter_context(tc.tile_pool(name="mstg", bufs=2))
        self.ystage = stack.enter_context(tc.tile_pool(name="ystage", bufs=2))

        self.xb = sb.tile([P, KD, S], BF16, name="xb")
        self.wq_sb = sb.tile([P, KD, DL], BF16, name="wq_sb")
        self.wk_sb = sb.tile([P, KD, DL], BF16, name="wk_sb")
        self.wv_sb = sb.tile([P, KD, DL], BF16, name="wv_sb")
        self.wfc_sb = sb.tile([P, DLC, D], BF16, name="wfc_sb")
        self.qT = sb.tile([P, DLC, S], BF16, name="qT")
        self.kT = sb.tile([P, DLC, S], BF16, name="kT")
        self.vN = sb.tile([P, NM, DL], BF16, name="vN")
        self.outT = sb.tile([P, DLC, S], BF16, name="outT")

        # loads: wq + x-block0 first (q0 projection starts earliest), then wk,
        # x blocks 1..3. sync+scalar are the HWDGE queues (gpsimd DMA goes via
        # the slow software DGE path - only wv/wfc, needed late, ride it).
        nc.scalar.dma_start(self.wq_sb[:], self.wq[:, :, :])
        for k in range(4):
            nc.sync.dma_start(self.xb[:, k, ds(0, SQ)], self.xT[:, k, ds(0, SQ)])
            nc.scalar.dma_start(self.xb[:, k + 4, ds(0, SQ)], self.xT[:, k + 4, ds(0, SQ)])
        nc.scalar.dma_start(self.wk_sb[:], self.wk[:, :, :])
        for k in range(4):
            nc.sync.dma_start(self.xb[:, k, ds(SQ, SQ)], self.xT[:, k, ds(SQ, SQ)])
            nc.scalar.dma_start(self.xb[:, k + 4, ds(SQ, SQ)], self.xT[:, k + 4, ds(SQ, SQ)])
        nc.gpsimd.dma_start(self.wv_sb[:], self.wv[:, :, :])
        for k in range(4):
            nc.sync.dma_start(self.xb[:, k, ds(S // 2, S // 2)], self.xT[:, k, ds(S // 2, S // 2)])
            nc.scalar.dma_start(self.xb[:, k + 4, ds(S // 2, S // 2)], self.xT[:, k + 4, ds(S // 2, S // 2)])
        nc.gpsimd.dma_start(self.wfc_sb[:], self.wfc[:, :, :])

        wk_sb, wq_sb = self.wk_sb, self.wq_sb
        K = lambda c, nb: self.kq_group(wk_sb, self.kT, c, nb)
        Q = lambda c, nb: self.kq_group(wq_sb, self.qT, c, nb)
        V = self.v_group
        FC = self.fc_group

        def fK(c, nb):
            return lambda: K(c, nb)
        def fQ(c, nb):
            return lambda: Q(c, nb)
        def fV(m):
            return lambda: V(m)
        def fFC(sc, eb):
            return lambda: FC(sc, eb)

        # upfront: q block 0 (pair), k(c0, nb0)
        self.cur_mask = self.load_mask(0)
        self.qp0()
        K(0, 0)

        # qb0 hp0: weave remaining k(c0) groups (gate scores quads), all v
        # groups (gate av quads), and the first k(c1) groups for hp1
        self.window(0, 0, [
            (0, [fK(0, 1)], None),
            (1, [fK(0, 2), fV(0)], None),
            (2, [fK(0, 3), fV(1), fV(2), fV(3)], 0),
            (3, [fV(4), fV(5), fV(6), fV(7)], 1),
            (None, [fK(1, 0), fV(8), fV(9), fV(10), fV(11)], 2),
            (None, [fK(1, 1), fV(12), fV(13), fV(14), fV(15)], 3),
        ])

        # qb0 hp1: fillers = remaining k(c1) groups + q block 1
        self.window(0, 1, [
            (0, [fQ(0, 1)], None),
            (1, [fK(1, 2)], None),
            (2, [fK(1, 3)], 0),
            (3, [fQ(1, 1)], 1),
            (None, [], 2),
            (None, [], 3),
        ])

        for qb in range(1, NQB):
            self.cur_mask = self.load_mask(qb)
            for hp in range(DLC):
                a = (qb - 1) * 4 + 2 * hp  # fc seq chunks for previous qb
                extra = [fQ(hp, qb + 1)] if qb < NQB - 1 else []
                self.window(qb, hp, [
                    (0, [fFC(a, 0)], None),
                    (1, [fFC(a, 1)], 0),
                    (2, [fFC(a + 1, 0)], 1),
                    (3, [fFC(a + 1, 1)] + extra, 2),
                    (None, [], 3),
                ])

        # tail: fc for the last qb
        for sc in range(12, 16):
            for eb in range(2):
                FC(sc, eb)

        stack.close()


# ---- host wrapper ---------------------------------------------------------

N_HEAD = 16
_nc_cache = {}


def get_nc(with_mask: bool):
    if with_mask not in _nc_cache:
        _nc_cache[with_mask] = build_nc(with_mask)
    return _nc_cache[with_mask]


def make_in_maps(x, mask, Wq, Wk, Wv, Wfc, with_mask):
    scale = np.float32(1.0 / np.sqrt(D // N_HEAD))
    bf = ml_dtypes.bfloat16
    in_maps = []
    for c in range(8):
        b, hg = divmod(c, 4)
        gs = slice(DL * hg, DL * hg + DL)
        def prearrange(wT, cdim):  # [cdim*128, F] -> [128, cdim, F]
            F = wT.shape[1]
            return np.ascontiguousarray(
                wT.reshape(cdim, P, F).transpose(1, 0, 2)
            ).astype(bf)

        m = {
            "xT": prearrange(x[b].T, KD),
            "wq": prearrange((Wq[gs, :] * scale).T, KD),
            "wk": prearrange(Wk[gs, :].T, KD),
            "wv": prearrange(Wv[gs, :].T, KD),
            "wfc": prearrange(Wfc[:, gs].T, DLC),
        }
        if with_mask:
            m["maskT"] = np.ascontiguousarray(
                np.broadcast_to(mask, (1, 1, S, S))[0, 0].T.astype(np.float32)
            )
        in_maps.append(m)
    return in_maps


def kernel(x, mask, Wq, Wk, Wv, Wfc, bfc):
    """Full-input entry: shards across 8 trn2 cores, returns the full output."""
    from concourse.bass_utils import run_bass_kernel_spmd

    x = np.asarray(x, dtype=np.float32)
    mask = np.asarray(mask, dtype=np.float32)
    Wq = np.asarray(Wq, dtype=np.float32)
    Wk = np.asarray(Wk, dtype=np.float32)
    Wv = np.asarray(Wv, dtype=np.float32)
    Wfc = np.asarray(Wfc, dtype=np.float32)
    bfc = np.asarray(bfc, dtype=np.float32)

    B = x.shape[0]
    with_mask = bool(np.any(mask))
    nc = get_nc(with_mask)
    in_maps = make_in_maps(x, mask, Wq, Wk, Wv, Wfc, with_mask)

    res = run_bass_kernel_spmd(nc, in_maps, core_ids=list(range(8)))
    parts = np.stack([np.asarray(r["y"]) for r in res.results])  # [8, S, D] bf16
    out = parts.astype(np.float64).reshape(B, 4, S, D).sum(axis=1)
    out += bfc.astype(np.float64)
    return out.astype(np.float32)
